# revision 52
# baseline (speedup 1.0000x reference)
"""EnvironmentalContextAttention on 8 trn2 NeuronCores.

Model (reference.py):
    q,k,v = heads(x@Wq+bq), heads(x@Wk+bk), heads(x@Wv+bv)      # [B,H,S,HD]
    scores = (q @ k^T) / sqrt(HD) * gate[b,h]                   # [B,H,S,S]
    gate   = sigmoid((env@We+be)@Wm+bm)                         # [B,H]
    out    = (softmax(scores) @ v).merge_heads() @ Wo + bo      # [B,S,D]

Sharding: 8 cores = 2 batches x 4 head-groups (4 heads each). Each core
computes its heads' attention and a partial out-projection (transposed,
[D, S]); the host sums the 4 partials per batch and re-transposes.

Exact device-side simplifications:
  * bk drops (per-query constant shift cancels in softmax).
  * gate/sqrt(HD) is folded into Wk columns ON THE HOST, so scores come
    out of the QK^T matmul pre-scaled and exp needs no scale operand.
  * bq is folded into qt during the Q-projection PSUM evacuation
    (tensor_scalar_add with a per-partition bq vector): k.(q+bq) equals
    k.q + k.bq, which is the exact bias the softmax needs.
  * bv, bo: softmax rows sum to 1 => host adds the constant row
    bv@Wo + bo once at the end.
  * softmax runs without the running-max shift (|gated scores| < ~8,
    exp cannot overflow fp32); sum-of-exp falls out of the attn@V
    matmul by appending a ones row to each head's V tile.

Dataflow per core, software-pipelined across engines:
  Head PAIRS (one head at partitions 0-63, one at 64-127) iterate over
  512-wide query chunks. Per key chunk both heads' score matmuls land
  in one [128, 2x512] PSUM tile — head-even via PE row-tile T0 into
  one bank, head-odd via T8 into the next (row tiles must write
  different banks; adjacent T0/T8 matmuls overlap ~2x, HW-measured).
  One plain-Exp ACT per key chunk covers both heads (~740ns — ScalarE
  runs 2x for bf16 output). Score tiles are double-buffered so TensorE
  never waits on ScalarE; scores for two key chunks are emitted back
  to back to halve PE tiling-mode switches. The previous key pair's
  ctx matmuls (lag-2) plus one queued projection/out-projection matmul
  group per step fill the remaining TensorE time — TensorE is the
  pacing engine. PSUM: 4 banks scores (2x2 double-buffered), 1 per
  head for ctx, 2 for the fill pipeline = 8 exactly.
  Normalization multiplies ctx rows (read straight from PSUM) by
  1/sumexp via reciprocal_approx_fast (SBUF-copied sum row) + gpsimd
  partition broadcast (gpsimd is broadcast-only so norm chains never
  queue behind DMA descriptor generation); out-projection partials
  leave as bf16 via the sync HWDGE queue, which is idle once the
  inputs have landed, and the host sums the 4 partials per batch in
  f32.
"""

import contextlib

import ml_dtypes
import numpy as np

import concourse.bass as bass
import concourse.mybir as mybir
from concourse import bacc
from concourse.tile import TileContext
from concourse import bass_utils

# problem constants (hardcoded per contract)
B, S, D, H, E = 2, 2048, 1024, 16, 256
HD = D // H            # 64
N_CORES = 8
HPC = H // 4           # 4 heads per core
J = HPC * HD           # 256 local columns
P = 128
KC = D // P            # 8 contraction chunks
TC = S // P            # 16 key chunks
JC = J // P            # 2 local j chunks
NC_O = D // P          # 8 output row chunks
QC = S // 512          # 4 query chunks of 512
NP = 2                 # head pairs per core

F32 = mybir.dt.float32
BF16 = mybir.dt.bfloat16
EXP = mybir.ActivationFunctionType.Exp
MULT = mybir.AluOpType.mult


def build_nc(repeats: int = 1, tiny_out: bool = False, upto: str = "full"):
    STAGES = ("proj", "exp", "ctx", "norm", "full")
    LVL = STAGES.index(upto)
    nc = bacc.Bacc("TRN2", target_bir_lowering=False, debug=False,
                   num_devices=N_CORES)

    xT = nc.dram_tensor("xT", [D, S], BF16, kind="ExternalInput").ap()
    wq = nc.dram_tensor("wq", [D, J], BF16, kind="ExternalInput").ap()
    wk = nc.dram_tensor("wk", [D, J], BF16, kind="ExternalInput").ap()
    wv = nc.dram_tensor("wv", [D, HPC * (HD + 1)], BF16, kind="ExternalInput").ap()
    wo = nc.dram_tensor("wo", [J, D], BF16, kind="ExternalInput").ap()
    bqr = nc.dram_tensor("bqr", [P, JC], F32, kind="ExternalInput").ap()
    if tiny_out:
        outT = nc.dram_tensor("outT", [D, S], BF16, kind="Internal").ap()
        tiny = nc.dram_tensor("tiny", [P, 512], F32, kind="ExternalOutput").ap()
    else:
        outT = nc.dram_tensor("outT", [D, S], BF16, kind="ExternalOutput").ap()
        tiny = None

    with TileContext(nc) as tc:
        with (
            tc.tile_pool(name="const", bufs=1) as const_pool,
            tc.tile_pool(name="xw", bufs=1) as xw_pool,
            tc.tile_pool(name="qkv", bufs=1) as qkv_pool,
            tc.tile_pool(name="etp", bufs=1) as et_pool,
            tc.tile_pool(name="work", bufs=1) as work_pool,
            tc.tile_pool(name="outsb", bufs=4) as out_pool,
            tc.tile_pool(name="ps_fill", bufs=2, space="PSUM") as ps_fill,
            tc.tile_pool(name="ps_sc", bufs=1, space="PSUM") as ps_sc,
            tc.tile_pool(name="ps_ctx", bufs=1, space="PSUM") as ps_ctx,
        ):
            bq_sb = const_pool.tile([P, JC], F32)
            nc.sync.dma_start(out=bq_sb[:], in_=bqr[:])

            rep_cm = (tc.For_i(0, repeats, 1) if repeats > 1
                      else contextlib.nullcontext())
            with rep_cm:
                # ---- resident inputs; DMA in consumption order ----
                # double-buffered across repeat iterations: the next
                # iteration's input DMAs land in the alternate buffer and
                # overlap this iteration's compute tail
                x_sb = xw_pool.tile([P, KC, S], BF16, name="x_sb", tag="x",
                                    bufs=2)
                wq_sb = xw_pool.tile([P, KC, J], BF16, name="wq_sb", tag="wq",
                                     bufs=2)
                wk_sb = xw_pool.tile([P, KC, J], BF16, name="wk_sb", tag="wk",
                                     bufs=2)
                wv_sb = xw_pool.tile([P, KC, HPC * (HD + 1)], BF16,
                                     name="wv_sb", tag="wv", bufs=2)
                wo_sb = xw_pool.tile([P, JC, D], BF16, name="wo_sb", tag="wo",
                                     bufs=2)
                # x (4MB) + wk gate the first matmuls: split the bulk x
                # transfer across both HWDGE queues (sync + scalar) and
                # push everything not on the critical path behind it
                xTr = xT.rearrange("(c p) s -> p c s", p=P)
                wqr = wq.rearrange("(c p) j -> p c j", p=P)
                wkr = wk.rearrange("(c p) j -> p c j", p=P)
                wvr = wv.rearrange("(c p) j -> p c j", p=P)
                # x lands in two waves of [128,1024] slices (2KB DRAM
                # rows - the DMA descriptor-throughput threshold): the
                # upfront projection groups only need the first wave, so
                # compute starts after 2MB instead of 4MB
                for half in range(2):
                    for k in range(KC):
                        eng = nc.sync if (k + half) % 2 == 0 else nc.scalar
                        eng.dma_start(
                            out=x_sb[:, k, half * 1024:(half + 1) * 1024],
                            in_=xTr[:, k, half * 1024:(half + 1) * 1024])
                for k in range(KC):
                    nc.sync.dma_start(out=wk_sb[:, k], in_=wkr[:, k])
                    nc.scalar.dma_start(out=wq_sb[:, k], in_=wqr[:, k])
                for k in range(KC):
                    nc.sync.dma_start(out=wv_sb[:, k], in_=wvr[:, k])
                nc.scalar.dma_start(out=wo_sb[:],
                                    in_=wo.rearrange("(c p) n -> p c n", p=P))

                qt_sb = qkv_pool.tile([P, JC, S], BF16, tag="qt", name="qt_sb")
                kt_sb = qkv_pool.tile([P, JC, S], BF16, tag="kt", name="kt_sb")
                v_sb = qkv_pool.tile([P, TC, HPC, HD + 1], BF16, tag="v",
                                     name="v_sb")
                ctn = qkv_pool.tile([P, JC, S], BF16, tag="ctn", name="ctn")

                # ---- fill-work generators (one TensorE burst each) ----
                def qk_group(dst, w_sb, jc, sc, is_q):
                    ps = ps_fill.tile([P, 512], F32, tag="fill", name="pf",
                                      bufs=2)
                    for k in range(KC):
                        nc.tensor.matmul(
                            ps[:],
                            lhsT=w_sb[:, k, jc * P:(jc + 1) * P],
                            rhs=x_sb[:, k, sc * 512:(sc + 1) * 512],
                            start=(k == 0), stop=(k == KC - 1),
                            skip_group_check=True,
                        )
                    dsts = dst[:, jc, sc * 512:(sc + 1) * 512]
                    if is_q:
                        nc.vector.tensor_scalar_add(
                            out=dsts, in0=ps[:], scalar1=bq_sb[:, jc:jc + 1])
                    else:
                        nc.vector.tensor_copy(out=dsts, in_=ps[:])

                def v_group(t):
                    ps = ps_fill.tile([P, 512], F32, tag="fill", name="pf",
                                      bufs=2)
                    pv = ps[:, :HPC * (HD + 1)]
                    for k in range(KC):
                        nc.tensor.matmul(
                            pv,
                            lhsT=x_sb[:, k, t * P:(t + 1) * P],
                            rhs=wv_sb[:, k, :],
                            start=(k == 0), stop=(k == KC - 1),
                            skip_group_check=True,
                        )
                    nc.vector.tensor_copy(out=v_sb[:, t], in_=pv)
                    nc.vector.memset(v_sb[:, t, :, HD:HD + 1], 1.0)

                def out_group(n, qh, tail=False):
                    if tail:
                        ps = ps_sc.tile([P, 2, 512], F32, tag="ps",
                                        name="ps", bufs=2)[:, 0, :]
                    else:
                        ps = ps_fill.tile([P, 512], F32, tag="fill",
                                          name="pf", bufs=2)
                    for jc in range(JC):
                        nc.tensor.matmul(
                            ps[:],
                            lhsT=wo_sb[:, jc, n * P:(n + 1) * P],
                            rhs=ctn[:, jc, qh * 512:(qh + 1) * 512],
                            start=(jc == 0), stop=(jc == JC - 1),
                            skip_group_check=True,
                        )
                    ot = out_pool.tile([P, 512], BF16, tag="ot", name="ot",
                                       bufs=3)
                    nc.vector.tensor_copy(out=ot[:], in_=ps[:])
                    nc.sync.dma_start(
                        out=outT[n * P:(n + 1) * P,
                                 qh * 512:(qh + 1) * 512],
                        in_=ot[:])

                # ordered queue of keyed fill closures; dependents pull
                # their prerequisites forward so TensorE never deadlocks
                # on a not-yet-emitted producer
                fills = {}
                for sc in range(1, QC):
                    fills[("kt", 0, sc)] = (
                        lambda sc=sc: qk_group(kt_sb, wk_sb, 0, sc, False))
                for t in range(1, TC):
                    fills[("v", t)] = lambda t=t: v_group(t)
                fills[("kt", 1, 0)] = (
                    lambda: qk_group(kt_sb, wk_sb, 1, 0, False))
                fills[("qt", 1, 0)] = (
                    lambda: qk_group(qt_sb, wq_sb, 1, 0, True))
                for sc in range(1, QC):
                    fills[("kt", 1, sc)] = (
                        lambda sc=sc: qk_group(kt_sb, wk_sb, 1, sc, False))
                    fills[("qt", 0, sc)] = (
                        lambda sc=sc: qk_group(qt_sb, wq_sb, 0, sc, True))
                    fills[("qt", 1, sc)] = (
                        lambda sc=sc: qk_group(qt_sb, wq_sb, 1, sc, True))

                def run_fill(key=None):
                    """Emit one queued fill group (a specific one, or the
                    oldest)."""
                    if key is None:
                        if fills:
                            fills.pop(next(iter(fills)))()
                    elif key in fills:
                        fills.pop(key)()

                # ---- upfront: K(jc0,cols 0-1023), Q(jc0,cols 0-1023),
                # V(0) ----
                qk_group(kt_sb, wk_sb, 0, 0, False)
                qk_group(qt_sb, wq_sb, 0, 0, True)
                v_group(0)

                # ---- attention: head pairs x query chunks ----
                # Per key-chunk t, both heads' scores land in ONE [128,
                # 2x512] PSUM tile: head-even via PE row-tile T0 into the
                # first bank, head-odd via T8 into the second (row tiles
                # must hit different banks; adjacent T0/T8 matmuls overlap
                # ~2x). One plain-Exp ACT covers both heads, and the tile
                # is double-buffered so TensorE never waits on ScalarE.
                def ctx_mms(p, t, qh):
                    """Both heads' ctx accumulation for key chunk t."""
                    for half in range(2):
                        cp = (ctx_e if half == 0 else ctx_o)
                        nc.tensor.matmul(
                            cp[:],
                            lhsT=v_sb[:, t, 2 * p + half, :],
                            rhs=ets[t][:, half, :],
                            start=(t == 0), stop=(t == TC - 1),
                            skip_group_check=True,
                        )

                for qh in range(QC if LVL >= 1 else 0):
                    for p in range(NP):
                        # prerequisite of this iteration's score matmuls
                        run_fill(("qt", p, qh))
                        ctx_e = ps_ctx.tile([HD + 1, 512], F32, tag="ctx_e",
                                            name="ctx_e", bufs=1)
                        ctx_o = ps_ctx.tile([HD + 1, 512], F32, tag="ctx_o",
                                            name="ctx_o", bufs=1)
                        ets = {}
                        for tp in range(TC // 2):
                            # scores for key chunks (2tp, 2tp+1) back to
                            # back: all four matmuls alternate the T0/T8
                            # row-tiles and only then does the array switch
                            # back to full-128 mode for ctx/fill work
                            pss = {}
                            for t in (2 * tp, 2 * tp + 1):
                                run_fill(("kt", p, t // 4))
                                pss[t] = ps_sc.tile([P, 2, 512], F32,
                                                    tag="ps", name="ps",
                                                    bufs=2)
                            for t in (2 * tp, 2 * tp + 1):
                                for half in range(2):
                                    lo = 64 * half
                                    nc.tensor.matmul(
                                        pss[t][:, half, :],
                                        lhsT=kt_sb[lo:lo + HD, p,
                                                   t * P:(t + 1) * P],
                                        rhs=qt_sb[lo:lo + HD, p,
                                                  qh * 512:(qh + 1) * 512],
                                        start=True, stop=True,
                                        skip_group_check=True,
                                    )
                            if tp > 0 and LVL >= 2:
                                for t in (2 * tp - 2, 2 * tp - 1):
                                    run_fill(("v", t))
                                    ctx_mms(p, t, qh)
                            run_fill()
                            for t in (2 * tp, 2 * tp + 1):
                                et = et_pool.tile([P, 2, 512], BF16,
                                                  tag="et", name="et",
                                                  bufs=4)
                                nc.scalar.activation(et[:], pss[t][:], EXP)
                                ets[t] = et
                        if LVL >= 2:
                            for t in (TC - 2, TC - 1):
                                run_fill(("v", t))
                                ctx_mms(p, t, qh)
                        if LVL < 3:
                            continue
                        # normalize: ctx rows stay in PSUM; the sum rows
                        # are copied to SBUF (reciprocal_approx_fast cannot
                        # read PSUM), inverted, broadcast, multiplied in.
                        # Both heads' chains are interleaved so the DVE and
                        # gpsimd steps overlap across heads.
                        cps = (ctx_e, ctx_o)
                        srs, rrs, pbs = [], [], []
                        for half, cp in enumerate(cps):
                            sr = work_pool.tile([1, 512], F32, tag=f"sr{half}",
                                                name="sr", bufs=2)
                            nc.vector.tensor_copy(out=sr[:],
                                                  in_=cp[HD:HD + 1, :])
                            srs.append(sr)
                        for half, cp in enumerate(cps):
                            rr = work_pool.tile([1, 512], F32, tag=f"rr{half}",
                                                name="rr", bufs=2)
                            nc.vector.reciprocal_approx_fast(
                                out=rr[:], in_=srs[half][:])
                            rrs.append(rr)
                        for half, cp in enumerate(cps):
                            pb = work_pool.tile([HD, 512], F32, tag=f"pb{half}",
                                                name="pb", bufs=2)
                            nc.gpsimd.partition_broadcast(pb[:], rrs[half][:])
                            pbs.append(pb)
                        for half, cp in enumerate(cps):
                            lo = 64 * half
                            nc.vector.tensor_tensor(
                                out=ctn[lo:lo + HD, p,
                                        qh * 512:(qh + 1) * 512],
                                in0=cp[0:HD, :], in1=pbs[half][:], op=MULT)
                        if p == 1 and LVL >= 4:
                            for n in range(NC_O):
                                fills[("out", n, qh)] = (
                                    lambda n=n, qh=qh: out_group(n, qh))

                for key in list(fills):
                    fn = fills.pop(key)
                    if key[0] == "out":
                        out_group(key[1], key[2], tail=True)
                    else:
                        fn()

            if tiny_out:
                tt = out_pool.tile([P, 512], F32, name="tt", tag="tt", bufs=1)
                nc.vector.memset(tt[:], 1.0)
                nc.sync.dma_start(out=tiny[:], in_=tt[:])

    nc.compile()
    return nc


_NC_CACHE = {}


def get_nc(repeats: int = 1, tiny_out: bool = False):
    key = (repeats, tiny_out)
    if key not in _NC_CACHE:
        _NC_CACHE[key] = build_nc(repeats, tiny_out)
    return _NC_CACHE[key]


def host_prep(inputs):
    """Shard + precompute per-core input maps; return (in_maps, out_bias_row)."""
    f = {k: np.asarray(v, dtype=np.float64) for k, v in inputs.items()}
    x, env = f["x"], f["env_context"]
    Wq, Wk, Wv, Wo = f["Wq"], f["Wk"], f["Wv"], f["Wo"]
    bq, bk, bv, bo = f["bq"], f["bk"], f["bv"], f["bo"]
    We, be, Wm, bm = f["We"], f["be"], f["Wm"], f["bm"]

    gate = 1.0 / (1.0 + np.exp(-((env @ We + be) @ Wm + bm)))  # [B, H]
    scale = gate / np.sqrt(HD)                                  # [B, H]

    in_maps = []
    for c in range(N_CORES):
        b, g = divmod(c, 4)
        cols = slice(J * g, J * (g + 1))
        # fold gate/sqrt(HD) into this core's Wk columns (per-column scale)
        wk_scaled = Wk[:, cols] * np.repeat(
            scale[b, HPC * g:HPC * (g + 1)], HD)[None, :]
        wv_pad = np.zeros((D, HPC * (HD + 1)), np.float64)
        for i in range(HPC):
            h = HPC * g + i
            wv_pad[:, i * (HD + 1):i * (HD + 1) + HD] = \
                Wv[:, HD * h:HD * (h + 1)]
        in_maps.append({
            "xT": np.ascontiguousarray(x[b].T).astype(ml_dtypes.bfloat16),
            "wq": np.ascontiguousarray(Wq[:, cols]).astype(ml_dtypes.bfloat16),
            "wk": np.ascontiguousarray(wk_scaled).astype(ml_dtypes.bfloat16),
            "wv": wv_pad.astype(np.float32).astype(ml_dtypes.bfloat16),
            "wo": np.ascontiguousarray(
                Wo[J * g:J * (g + 1), :]).astype(ml_dtypes.bfloat16),
            "bqr": np.ascontiguousarray(
                bq[cols].reshape(JC, P).T).astype(np.float32),
        })
    out_bias_row = (bv @ Wo + bo).astype(np.float32)  # [D]
    return in_maps, out_bias_row


def assemble(results, out_bias_row):
    out = np.zeros((B, S, D), np.float32)
    for c in range(N_CORES):
        b = c // 4
        out[b] += results[c]["outT"].T.astype(np.float32)
    out += out_bias_row[None, None, :]
    return out


def kernel(**inputs):
    import time as _time

    nc = get_nc(1)
    in_maps, out_bias_row = host_prep(inputs)
    last_err = None
    for _attempt in range(4):
        try:
            res = bass_utils.run_bass_kernel_spmd(
                nc, in_maps, core_ids=list(range(N_CORES)))
            return assemble(res.results, out_bias_row)
        except Exception as e:  # transient NRT/axon hiccups recover on retry
            last_err = e
            _time.sleep(5)
    raise last_err


# revision 53
# speedup vs baseline: 1.0211x; 1.0211x over previous
"""EnvironmentalContextAttention on 8 trn2 NeuronCores.

Model (reference.py):
    q,k,v = heads(x@Wq+bq), heads(x@Wk+bk), heads(x@Wv+bv)      # [B,H,S,HD]
    scores = (q @ k^T) / sqrt(HD) * gate[b,h]                   # [B,H,S,S]
    gate   = sigmoid((env@We+be)@Wm+bm)                         # [B,H]
    out    = (softmax(scores) @ v).merge_heads() @ Wo + bo      # [B,S,D]

Sharding: 8 cores = 2 batches x 4 head-groups (4 heads each). Each core
computes its heads' attention and a partial out-projection (transposed,
[D, S]); the host sums the 4 partials per batch and re-transposes.

Exact device-side simplifications:
  * bk drops (per-query constant shift cancels in softmax).
  * gate/sqrt(HD) is folded into Wk columns ON THE HOST, so scores come
    out of the QK^T matmul pre-scaled and exp needs no scale operand.
  * bq is folded into qt during the Q-projection PSUM evacuation
    (tensor_scalar_add with a per-partition bq vector): k.(q+bq) equals
    k.q + k.bq, which is the exact bias the softmax needs.
  * bv, bo: softmax rows sum to 1 => host adds the constant row
    bv@Wo + bo once at the end.
  * softmax runs without the running-max shift (|gated scores| < ~8,
    exp cannot overflow fp32); sum-of-exp falls out of the attn@V
    matmul by appending a ones row to each head's V tile.

Dataflow per core, software-pipelined across engines:
  Head PAIRS (one head at partitions 0-63, one at 64-127) iterate over
  512-wide query chunks. Per key chunk both heads' score matmuls land
  in one [128, 2x512] PSUM tile — head-even via PE row-tile T0 into
  one bank, head-odd via T8 into the next (row tiles must write
  different banks; adjacent T0/T8 matmuls overlap ~2x, HW-measured).
  One plain-Exp ACT per key chunk covers both heads (~740ns — ScalarE
  runs 2x for bf16 output). Score tiles are double-buffered so TensorE
  never waits on ScalarE; scores for two key chunks are emitted back
  to back to halve PE tiling-mode switches. The previous key pair's
  ctx matmuls (lag-2) plus one queued projection/out-projection matmul
  group per step fill the remaining TensorE time — TensorE is the
  pacing engine. PSUM: 4 banks scores (2x2 double-buffered), 1 per
  head for ctx, 2 for the fill pipeline = 8 exactly.
  Normalization multiplies ctx rows (read straight from PSUM) by
  1/sumexp via reciprocal_approx_fast (SBUF-copied sum row) + gpsimd
  partition broadcast (gpsimd is broadcast-only so norm chains never
  queue behind DMA descriptor generation); out-projection partials
  leave as bf16 via the sync HWDGE queue, which is idle once the
  inputs have landed, and the host sums the 4 partials per batch in
  f32.
"""

import contextlib

import ml_dtypes
import numpy as np

import concourse.bass as bass
import concourse.mybir as mybir
from concourse import bacc
from concourse.tile import TileContext
from concourse import bass_utils

# problem constants (hardcoded per contract)
B, S, D, H, E = 2, 2048, 1024, 16, 256
HD = D // H            # 64
N_CORES = 8
HPC = H // 4           # 4 heads per core
J = HPC * HD           # 256 local columns
P = 128
KC = D // P            # 8 contraction chunks
TC = S // P            # 16 key chunks
JC = J // P            # 2 local j chunks
NC_O = D // P          # 8 output row chunks
QC = S // 512          # 4 query chunks of 512
NP = 2                 # head pairs per core

F32 = mybir.dt.float32
BF16 = mybir.dt.bfloat16
EXP = mybir.ActivationFunctionType.Exp
MULT = mybir.AluOpType.mult


def build_nc(repeats: int = 1, tiny_out: bool = False, upto: str = "full"):
    STAGES = ("proj", "exp", "ctx", "norm", "full")
    LVL = STAGES.index(upto)
    nc = bacc.Bacc("TRN2", target_bir_lowering=False, debug=False,
                   num_devices=N_CORES)

    xT = nc.dram_tensor("xT", [D, S], BF16, kind="ExternalInput").ap()
    wq = nc.dram_tensor("wq", [D, J], BF16, kind="ExternalInput").ap()
    wk = nc.dram_tensor("wk", [D, J], BF16, kind="ExternalInput").ap()
    wv = nc.dram_tensor("wv", [D, HPC * (HD + 1)], BF16, kind="ExternalInput").ap()
    wo = nc.dram_tensor("wo", [J, D], BF16, kind="ExternalInput").ap()
    bqr = nc.dram_tensor("bqr", [P, JC], F32, kind="ExternalInput").ap()
    if tiny_out:
        outT = nc.dram_tensor("outT", [D, S], BF16, kind="Internal").ap()
        tiny = nc.dram_tensor("tiny", [P, 512], F32, kind="ExternalOutput").ap()
    else:
        outT = nc.dram_tensor("outT", [D, S], BF16, kind="ExternalOutput").ap()
        tiny = None

    with TileContext(nc) as tc:
        with (
            tc.tile_pool(name="const", bufs=1) as const_pool,
            tc.tile_pool(name="xw", bufs=1) as xw_pool,
            tc.tile_pool(name="qkv", bufs=1) as qkv_pool,
            tc.tile_pool(name="etp", bufs=1) as et_pool,
            tc.tile_pool(name="work", bufs=1) as work_pool,
            tc.tile_pool(name="outsb", bufs=4) as out_pool,
            tc.tile_pool(name="ps_fill", bufs=2, space="PSUM") as ps_fill,
            tc.tile_pool(name="ps_sc", bufs=1, space="PSUM") as ps_sc,
            tc.tile_pool(name="ps_ctx", bufs=1, space="PSUM") as ps_ctx,
        ):
            bq_sb = const_pool.tile([P, JC], F32)
            nc.sync.dma_start(out=bq_sb[:], in_=bqr[:])

            rep_cm = (tc.For_i(0, repeats, 1) if repeats > 1
                      else contextlib.nullcontext())
            with rep_cm:
                # ---- resident inputs; DMA in consumption order ----
                # double-buffered across repeat iterations: the next
                # iteration's input DMAs land in the alternate buffer and
                # overlap this iteration's compute tail
                x_sb = xw_pool.tile([P, KC, S], BF16, name="x_sb", tag="x",
                                    bufs=2)
                wq_sb = xw_pool.tile([P, KC, J], BF16, name="wq_sb", tag="wq",
                                     bufs=2)
                wk_sb = xw_pool.tile([P, KC, J], BF16, name="wk_sb", tag="wk",
                                     bufs=2)
                wv_sb = xw_pool.tile([P, KC, HPC * (HD + 1)], BF16,
                                     name="wv_sb", tag="wv", bufs=2)
                wo_sb = xw_pool.tile([P, JC, D], BF16, name="wo_sb", tag="wo",
                                     bufs=2)
                # x (4MB) + wk gate the first matmuls: split the bulk x
                # transfer across both HWDGE queues (sync + scalar) and
                # push everything not on the critical path behind it
                xTr = xT.rearrange("(c p) s -> p c s", p=P)
                wqr = wq.rearrange("(c p) j -> p c j", p=P)
                wkr = wk.rearrange("(c p) j -> p c j", p=P)
                wvr = wv.rearrange("(c p) j -> p c j", p=P)
                # x lands in two waves of [128,1024] slices (2KB DRAM
                # rows - the DMA descriptor-throughput threshold): the
                # upfront projection groups only need the first wave, so
                # compute starts after 2MB instead of 4MB
                for half in range(2):
                    for k in range(KC):
                        eng = nc.sync if (k + half) % 2 == 0 else nc.scalar
                        eng.dma_start(
                            out=x_sb[:, k, half * 1024:(half + 1) * 1024],
                            in_=xTr[:, k, half * 1024:(half + 1) * 1024])
                for k in range(KC):
                    nc.sync.dma_start(out=wk_sb[:, k], in_=wkr[:, k])
                    nc.scalar.dma_start(out=wq_sb[:, k], in_=wqr[:, k])
                for k in range(KC):
                    nc.sync.dma_start(out=wv_sb[:, k], in_=wvr[:, k])
                nc.scalar.dma_start(out=wo_sb[:],
                                    in_=wo.rearrange("(c p) n -> p c n", p=P))

                qt_sb = qkv_pool.tile([P, JC, S], BF16, tag="qt", name="qt_sb")
                kt_sb = qkv_pool.tile([P, JC, S], BF16, tag="kt", name="kt_sb")
                v_sb = qkv_pool.tile([P, TC, HPC, HD + 1], BF16, tag="v",
                                     name="v_sb")
                ctn = qkv_pool.tile([P, JC, S], BF16, tag="ctn", name="ctn")

                # ---- fill-work generators (one TensorE burst each) ----
                def qk_group(dst, w_sb, jc, sc, is_q):
                    ps = ps_fill.tile([P, 512], F32, tag="fill", name="pf",
                                      bufs=2)
                    for k in range(KC):
                        nc.tensor.matmul(
                            ps[:],
                            lhsT=w_sb[:, k, jc * P:(jc + 1) * P],
                            rhs=x_sb[:, k, sc * 512:(sc + 1) * 512],
                            start=(k == 0), stop=(k == KC - 1),
                            skip_group_check=True,
                        )
                    dsts = dst[:, jc, sc * 512:(sc + 1) * 512]
                    if is_q:
                        nc.vector.tensor_scalar_add(
                            out=dsts, in0=ps[:], scalar1=bq_sb[:, jc:jc + 1])
                    else:
                        nc.vector.tensor_copy(out=dsts, in_=ps[:])

                def v_group(t):
                    ps = ps_fill.tile([P, 512], F32, tag="fill", name="pf",
                                      bufs=2)
                    pv = ps[:, :HPC * (HD + 1)]
                    for k in range(KC):
                        nc.tensor.matmul(
                            pv,
                            lhsT=x_sb[:, k, t * P:(t + 1) * P],
                            rhs=wv_sb[:, k, :],
                            start=(k == 0), stop=(k == KC - 1),
                            skip_group_check=True,
                        )
                    nc.vector.tensor_copy(out=v_sb[:, t], in_=pv)
                    nc.vector.memset(v_sb[:, t, :, HD:HD + 1], 1.0)

                def out_group(n, qh, tail=False):
                    if tail:
                        ps = ps_sc.tile([P, 2, 512], F32, tag="ps",
                                        name="ps", bufs=2)[:, 0, :]
                    else:
                        ps = ps_fill.tile([P, 512], F32, tag="fill",
                                          name="pf", bufs=2)
                    for jc in range(JC):
                        nc.tensor.matmul(
                            ps[:],
                            lhsT=wo_sb[:, jc, n * P:(n + 1) * P],
                            rhs=ctn[:, jc, qh * 512:(qh + 1) * 512],
                            start=(jc == 0), stop=(jc == JC - 1),
                            skip_group_check=True,
                        )
                    ot = out_pool.tile([P, 512], BF16, tag="ot", name="ot",
                                       bufs=3)
                    nc.vector.tensor_copy(out=ot[:], in_=ps[:])
                    nc.sync.dma_start(
                        out=outT[n * P:(n + 1) * P,
                                 qh * 512:(qh + 1) * 512],
                        in_=ot[:])

                # ordered queue of keyed fill closures; dependents pull
                # their prerequisites forward so TensorE never deadlocks
                # on a not-yet-emitted producer
                fills = {}
                for sc in range(1, QC):
                    fills[("kt", 0, sc)] = (
                        lambda sc=sc: qk_group(kt_sb, wk_sb, 0, sc, False))
                for t in range(1, TC):
                    fills[("v", t)] = lambda t=t: v_group(t)
                for sc in range(1, QC):
                    fills[("qt", 0, sc)] = (
                        lambda sc=sc: qk_group(qt_sb, wq_sb, 0, sc, True))
                for sc in range(QC):
                    fills[("kt", 1, sc)] = (
                        lambda sc=sc: qk_group(kt_sb, wk_sb, 1, sc, False))
                for sc in range(QC):
                    fills[("qt", 1, sc)] = (
                        lambda sc=sc: qk_group(qt_sb, wq_sb, 1, sc, True))

                def run_fill(key=None):
                    """Emit one queued fill group (a specific one, or the
                    oldest)."""
                    if key is None:
                        if fills:
                            fills.pop(next(iter(fills)))()
                    elif key in fills:
                        fills.pop(key)()

                # ---- upfront: K(jc0,cols 0-1023), Q(jc0,cols 0-1023),
                # V(0) ----
                qk_group(kt_sb, wk_sb, 0, 0, False)
                qk_group(qt_sb, wq_sb, 0, 0, True)
                v_group(0)

                # ---- attention: head pairs x query chunks ----
                # Per key-chunk t, both heads' scores land in ONE [128,
                # 2x512] PSUM tile: head-even via PE row-tile T0 into the
                # first bank, head-odd via T8 into the second (row tiles
                # must hit different banks; adjacent T0/T8 matmuls overlap
                # ~2x). One plain-Exp ACT covers both heads, and the tile
                # is double-buffered so TensorE never waits on ScalarE.
                def ctx_mms(p, t, qh):
                    """Both heads' ctx accumulation for key chunk t."""
                    for half in range(2):
                        cp = (ctx_e if half == 0 else ctx_o)
                        nc.tensor.matmul(
                            cp[:],
                            lhsT=v_sb[:, t, 2 * p + half, :],
                            rhs=ets[t][:, half, :],
                            start=(t == 0), stop=(t == TC - 1),
                            skip_group_check=True,
                        )

                for p in range(NP if LVL >= 1 else 0):
                    for qh in range(QC):
                        # prerequisite of this iteration's score matmuls
                        run_fill(("qt", p, qh))
                        ctx_e = ps_ctx.tile([HD + 1, 512], F32, tag="ctx_e",
                                            name="ctx_e", bufs=1)
                        ctx_o = ps_ctx.tile([HD + 1, 512], F32, tag="ctx_o",
                                            name="ctx_o", bufs=1)
                        ets = {}
                        for tp in range(TC // 2):
                            # scores for key chunks (2tp, 2tp+1) back to
                            # back: all four matmuls alternate the T0/T8
                            # row-tiles and only then does the array switch
                            # back to full-128 mode for ctx/fill work
                            pss = {}
                            for t in (2 * tp, 2 * tp + 1):
                                run_fill(("kt", p, t // 4))
                                pss[t] = ps_sc.tile([P, 2, 512], F32,
                                                    tag="ps", name="ps",
                                                    bufs=2)
                            for t in (2 * tp, 2 * tp + 1):
                                for half in range(2):
                                    lo = 64 * half
                                    nc.tensor.matmul(
                                        pss[t][:, half, :],
                                        lhsT=kt_sb[lo:lo + HD, p,
                                                   t * P:(t + 1) * P],
                                        rhs=qt_sb[lo:lo + HD, p,
                                                  qh * 512:(qh + 1) * 512],
                                        start=True, stop=True,
                                        skip_group_check=True,
                                    )
                            if tp > 0 and LVL >= 2:
                                for t in (2 * tp - 2, 2 * tp - 1):
                                    run_fill(("v", t))
                                    ctx_mms(p, t, qh)
                            run_fill()
                            for t in (2 * tp, 2 * tp + 1):
                                et = et_pool.tile([P, 2, 512], BF16,
                                                  tag="et", name="et",
                                                  bufs=4)
                                nc.scalar.activation(et[:], pss[t][:], EXP)
                                ets[t] = et
                        if LVL >= 2:
                            for t in (TC - 2, TC - 1):
                                run_fill(("v", t))
                                ctx_mms(p, t, qh)
                        if LVL < 3:
                            continue
                        # normalize: ctx rows stay in PSUM; the sum rows
                        # are copied to SBUF (reciprocal_approx_fast cannot
                        # read PSUM), inverted, broadcast, multiplied in.
                        # Both heads' chains are interleaved so the DVE and
                        # gpsimd steps overlap across heads.
                        cps = (ctx_e, ctx_o)
                        srs, rrs, pbs = [], [], []
                        for half, cp in enumerate(cps):
                            sr = work_pool.tile([1, 512], F32, tag=f"sr{half}",
                                                name="sr", bufs=2)
                            nc.vector.tensor_copy(out=sr[:],
                                                  in_=cp[HD:HD + 1, :])
                            srs.append(sr)
                        for half, cp in enumerate(cps):
                            rr = work_pool.tile([1, 512], F32, tag=f"rr{half}",
                                                name="rr", bufs=2)
                            nc.vector.reciprocal_approx_fast(
                                out=rr[:], in_=srs[half][:])
                            rrs.append(rr)
                        for half, cp in enumerate(cps):
                            pb = work_pool.tile([HD, 512], F32, tag=f"pb{half}",
                                                name="pb", bufs=2)
                            nc.gpsimd.partition_broadcast(pb[:], rrs[half][:])
                            pbs.append(pb)
                        for half, cp in enumerate(cps):
                            lo = 64 * half
                            nc.vector.tensor_tensor(
                                out=ctn[lo:lo + HD, p,
                                        qh * 512:(qh + 1) * 512],
                                in0=cp[0:HD, :], in1=pbs[half][:], op=MULT)
                        if p == 1 and LVL >= 4:
                            for n in range(NC_O):
                                fills[("out", n, qh)] = (
                                    lambda n=n, qh=qh: out_group(n, qh))

                for key in list(fills):
                    fn = fills.pop(key)
                    if key[0] == "out":
                        out_group(key[1], key[2], tail=True)
                    else:
                        fn()

            if tiny_out:
                tt = out_pool.tile([P, 512], F32, name="tt", tag="tt", bufs=1)
                nc.vector.memset(tt[:], 1.0)
                nc.sync.dma_start(out=tiny[:], in_=tt[:])

    nc.compile()
    return nc


_NC_CACHE = {}


def get_nc(repeats: int = 1, tiny_out: bool = False):
    key = (repeats, tiny_out)
    if key not in _NC_CACHE:
        _NC_CACHE[key] = build_nc(repeats, tiny_out)
    return _NC_CACHE[key]


def host_prep(inputs):
    """Shard + precompute per-core input maps; return (in_maps, out_bias_row)."""
    f = {k: np.asarray(v, dtype=np.float64) for k, v in inputs.items()}
    x, env = f["x"], f["env_context"]
    Wq, Wk, Wv, Wo = f["Wq"], f["Wk"], f["Wv"], f["Wo"]
    bq, bk, bv, bo = f["bq"], f["bk"], f["bv"], f["bo"]
    We, be, Wm, bm = f["We"], f["be"], f["Wm"], f["bm"]

    gate = 1.0 / (1.0 + np.exp(-((env @ We + be) @ Wm + bm)))  # [B, H]
    scale = gate / np.sqrt(HD)                                  # [B, H]

    in_maps = []
    for c in range(N_CORES):
        b, g = divmod(c, 4)
        cols = slice(J * g, J * (g + 1))
        # fold gate/sqrt(HD) into this core's Wk columns (per-column scale)
        wk_scaled = Wk[:, cols] * np.repeat(
            scale[b, HPC * g:HPC * (g + 1)], HD)[None, :]
        wv_pad = np.zeros((D, HPC * (HD + 1)), np.float64)
        for i in range(HPC):
            h = HPC * g + i
            wv_pad[:, i * (HD + 1):i * (HD + 1) + HD] = \
                Wv[:, HD * h:HD * (h + 1)]
        in_maps.append({
            "xT": np.ascontiguousarray(x[b].T).astype(ml_dtypes.bfloat16),
            "wq": np.ascontiguousarray(Wq[:, cols]).astype(ml_dtypes.bfloat16),
            "wk": np.ascontiguousarray(wk_scaled).astype(ml_dtypes.bfloat16),
            "wv": wv_pad.astype(np.float32).astype(ml_dtypes.bfloat16),
            "wo": np.ascontiguousarray(
                Wo[J * g:J * (g + 1), :]).astype(ml_dtypes.bfloat16),
            "bqr": np.ascontiguousarray(
                bq[cols].reshape(JC, P).T).astype(np.float32),
        })
    out_bias_row = (bv @ Wo + bo).astype(np.float32)  # [D]
    return in_maps, out_bias_row


def assemble(results, out_bias_row):
    out = np.zeros((B, S, D), np.float32)
    for c in range(N_CORES):
        b = c // 4
        out[b] += results[c]["outT"].T.astype(np.float32)
    out += out_bias_row[None, None, :]
    return out


def kernel(**inputs):
    import time as _time

    nc = get_nc(1)
    in_maps, out_bias_row = host_prep(inputs)
    last_err = None
    for _attempt in range(4):
        try:
            res = bass_utils.run_bass_kernel_spmd(
                nc, in_maps, core_ids=list(range(N_CORES)))
            return assemble(res.results, out_bias_row)
        except Exception as e:  # transient NRT/axon hiccups recover on retry
            last_err = e
            _time.sleep(5)
    raise last_err


# revision 55
# speedup vs baseline: 1.0231x; 1.0020x over previous
"""EnvironmentalContextAttention on 8 trn2 NeuronCores.

Model (reference.py):
    q,k,v = heads(x@Wq+bq), heads(x@Wk+bk), heads(x@Wv+bv)      # [B,H,S,HD]
    scores = (q @ k^T) / sqrt(HD) * gate[b,h]                   # [B,H,S,S]
    gate   = sigmoid((env@We+be)@Wm+bm)                         # [B,H]
    out    = (softmax(scores) @ v).merge_heads() @ Wo + bo      # [B,S,D]

Sharding: 8 cores = 2 batches x 4 head-groups (4 heads each). Each core
computes its heads' attention and a partial out-projection (transposed,
[D, S]); the host sums the 4 partials per batch and re-transposes.

Exact device-side simplifications:
  * bk drops (per-query constant shift cancels in softmax).
  * gate/sqrt(HD) is folded into Wk columns ON THE HOST, so scores come
    out of the QK^T matmul pre-scaled and exp needs no scale operand.
  * bq is folded into qt during the Q-projection PSUM evacuation
    (tensor_scalar_add with a per-partition bq vector): k.(q+bq) equals
    k.q + k.bq, which is the exact bias the softmax needs.
  * bv, bo: softmax rows sum to 1 => host adds the constant row
    bv@Wo + bo once at the end.
  * softmax runs without the running-max shift (|gated scores| < ~8,
    exp cannot overflow fp32); sum-of-exp falls out of the attn@V
    matmul by appending a ones row to each head's V tile.

Dataflow per core, software-pipelined across engines:
  Head PAIRS (one head at partitions 0-63, one at 64-127) iterate over
  512-wide query chunks. Per key chunk both heads' score matmuls land
  in one [128, 2x512] PSUM tile — head-even via PE row-tile T0 into
  one bank, head-odd via T8 into the next (row tiles must write
  different banks; adjacent T0/T8 matmuls overlap ~2x, HW-measured).
  One plain-Exp ACT per key chunk covers both heads (~740ns — ScalarE
  runs 2x for bf16 output). Score tiles are double-buffered so TensorE
  never waits on ScalarE; scores for two key chunks are emitted back
  to back to halve PE tiling-mode switches. The previous key pair's
  ctx matmuls (lag-2) plus one queued projection/out-projection matmul
  group per step fill the remaining TensorE time — TensorE is the
  pacing engine. PSUM: 4 banks scores (2x2 double-buffered), 1 per
  head for ctx, 2 for the fill pipeline = 8 exactly.
  Normalization multiplies ctx rows (read straight from PSUM) by
  1/sumexp via reciprocal_approx_fast (SBUF-copied sum row) + gpsimd
  partition broadcast (gpsimd is broadcast-only so norm chains never
  queue behind DMA descriptor generation); out-projection partials
  leave as bf16 via the sync HWDGE queue, which is idle once the
  inputs have landed, and the host sums the 4 partials per batch in
  f32.
"""

import contextlib

import ml_dtypes
import numpy as np

import concourse.bass as bass
import concourse.mybir as mybir
from concourse import bacc
from concourse.tile import TileContext
from concourse import bass_utils

# problem constants (hardcoded per contract)
B, S, D, H, E = 2, 2048, 1024, 16, 256
HD = D // H            # 64
N_CORES = 8
HPC = H // 4           # 4 heads per core
J = HPC * HD           # 256 local columns
P = 128
KC = D // P            # 8 contraction chunks
TC = S // P            # 16 key chunks
JC = J // P            # 2 local j chunks
NC_O = D // P          # 8 output row chunks
QC = S // 512          # 4 query chunks of 512
NP = 2                 # head pairs per core

F32 = mybir.dt.float32
BF16 = mybir.dt.bfloat16
EXP = mybir.ActivationFunctionType.Exp
MULT = mybir.AluOpType.mult


def build_nc(repeats: int = 1, tiny_out: bool = False, upto: str = "full"):
    STAGES = ("proj", "exp", "ctx", "norm", "full")
    LVL = STAGES.index(upto)
    nc = bacc.Bacc("TRN2", target_bir_lowering=False, debug=False,
                   num_devices=N_CORES)

    xT = nc.dram_tensor("xT", [D, S], BF16, kind="ExternalInput").ap()
    wq = nc.dram_tensor("wq", [D, J], BF16, kind="ExternalInput").ap()
    wk = nc.dram_tensor("wk", [D, J], BF16, kind="ExternalInput").ap()
    wv = nc.dram_tensor("wv", [D, HPC * (HD + 1)], BF16, kind="ExternalInput").ap()
    wo = nc.dram_tensor("wo", [J, D], BF16, kind="ExternalInput").ap()
    bqr = nc.dram_tensor("bqr", [P, JC], F32, kind="ExternalInput").ap()
    if tiny_out:
        outT = nc.dram_tensor("outT", [D, S], BF16, kind="Internal").ap()
        tiny = nc.dram_tensor("tiny", [P, 512], F32, kind="ExternalOutput").ap()
    else:
        outT = nc.dram_tensor("outT", [D, S], BF16, kind="ExternalOutput").ap()
        tiny = None

    with TileContext(nc) as tc:
        with (
            tc.tile_pool(name="const", bufs=1) as const_pool,
            tc.tile_pool(name="xw", bufs=1) as xw_pool,
            tc.tile_pool(name="qkv", bufs=1) as qkv_pool,
            tc.tile_pool(name="etp", bufs=1) as et_pool,
            tc.tile_pool(name="work", bufs=1) as work_pool,
            tc.tile_pool(name="outsb", bufs=4) as out_pool,
            tc.tile_pool(name="ps_fill", bufs=2, space="PSUM") as ps_fill,
            tc.tile_pool(name="ps_sc", bufs=1, space="PSUM") as ps_sc,
            tc.tile_pool(name="ps_ctx", bufs=1, space="PSUM") as ps_ctx,
        ):
            bq_sb = const_pool.tile([P, JC], F32)
            nc.sync.dma_start(out=bq_sb[:], in_=bqr[:])

            rep_cm = (tc.For_i(0, repeats, 1) if repeats > 1
                      else contextlib.nullcontext())
            with rep_cm:
                # ---- resident inputs; DMA in consumption order ----
                # double-buffered across repeat iterations: the next
                # iteration's input DMAs land in the alternate buffer and
                # overlap this iteration's compute tail
                x_sb = xw_pool.tile([P, KC, S], BF16, name="x_sb", tag="x",
                                    bufs=2)
                wq_sb = xw_pool.tile([P, KC, J], BF16, name="wq_sb", tag="wq",
                                     bufs=2)
                wk_sb = xw_pool.tile([P, KC, J], BF16, name="wk_sb", tag="wk",
                                     bufs=2)
                wv_sb = xw_pool.tile([P, KC, HPC * (HD + 1)], BF16,
                                     name="wv_sb", tag="wv", bufs=2)
                wo_sb = xw_pool.tile([P, JC, D], BF16, name="wo_sb", tag="wo",
                                     bufs=2)
                # x (4MB) + wk gate the first matmuls: split the bulk x
                # transfer across both HWDGE queues (sync + scalar) and
                # push everything not on the critical path behind it
                xTr = xT.rearrange("(c p) s -> p c s", p=P)
                wqr = wq.rearrange("(c p) j -> p c j", p=P)
                wkr = wk.rearrange("(c p) j -> p c j", p=P)
                wvr = wv.rearrange("(c p) j -> p c j", p=P)
                # x lands in two waves of [128,1024] slices (2KB DRAM
                # rows - the DMA descriptor-throughput threshold): the
                # upfront projection groups only need the first wave, so
                # compute starts after 2MB instead of 4MB
                for half in range(2):
                    for k in range(KC):
                        eng = nc.sync if (k + half) % 2 == 0 else nc.scalar
                        eng.dma_start(
                            out=x_sb[:, k, half * 1024:(half + 1) * 1024],
                            in_=xTr[:, k, half * 1024:(half + 1) * 1024])
                for k in range(KC):
                    nc.sync.dma_start(out=wk_sb[:, k], in_=wkr[:, k])
                    nc.scalar.dma_start(out=wq_sb[:, k], in_=wqr[:, k])
                for k in range(KC):
                    nc.sync.dma_start(out=wv_sb[:, k], in_=wvr[:, k])
                nc.scalar.dma_start(out=wo_sb[:],
                                    in_=wo.rearrange("(c p) n -> p c n", p=P))

                qt_sb = qkv_pool.tile([P, JC, S], BF16, tag="qt", name="qt_sb")
                kt_sb = qkv_pool.tile([P, JC, S], BF16, tag="kt", name="kt_sb")
                v_sb = qkv_pool.tile([P, TC, HPC, HD + 1], BF16, tag="v",
                                     name="v_sb")
                ctn = qkv_pool.tile([P, JC, S], BF16, tag="ctn", name="ctn")

                # ---- fill-work generators (one TensorE burst each) ----
                def qk_group(dst, w_sb, jc, sc, is_q):
                    ps = ps_fill.tile([P, 512], F32, tag="fill", name="pf",
                                      bufs=2)
                    for k in range(KC):
                        nc.tensor.matmul(
                            ps[:],
                            lhsT=w_sb[:, k, jc * P:(jc + 1) * P],
                            rhs=x_sb[:, k, sc * 512:(sc + 1) * 512],
                            start=(k == 0), stop=(k == KC - 1),
                            skip_group_check=True,
                        )
                    dsts = dst[:, jc, sc * 512:(sc + 1) * 512]
                    if is_q:
                        nc.vector.tensor_scalar_add(
                            out=dsts, in0=ps[:], scalar1=bq_sb[:, jc:jc + 1])
                    else:
                        nc.vector.tensor_copy(out=dsts, in_=ps[:])

                def v_group(t):
                    ps = ps_fill.tile([P, 512], F32, tag="fill", name="pf",
                                      bufs=2)
                    pv = ps[:, :HPC * (HD + 1)]
                    for k in range(KC):
                        nc.tensor.matmul(
                            pv,
                            lhsT=x_sb[:, k, t * P:(t + 1) * P],
                            rhs=wv_sb[:, k, :],
                            start=(k == 0), stop=(k == KC - 1),
                            skip_group_check=True,
                        )
                    nc.vector.tensor_copy(out=v_sb[:, t], in_=pv)
                    nc.vector.memset(v_sb[:, t, :, HD:HD + 1], 1.0)

                def out_group(n, qh, tail=False):
                    if tail:
                        ps = ps_sc.tile([P, 2, 512], F32, tag="ps",
                                        name="ps", bufs=2)[:, 0, :]
                    else:
                        ps = ps_fill.tile([P, 512], F32, tag="fill",
                                          name="pf", bufs=2)
                    for jc in range(JC):
                        nc.tensor.matmul(
                            ps[:],
                            lhsT=wo_sb[:, jc, n * P:(n + 1) * P],
                            rhs=ctn[:, jc, qh * 512:(qh + 1) * 512],
                            start=(jc == 0), stop=(jc == JC - 1),
                            skip_group_check=True,
                        )
                    ot = out_pool.tile([P, 512], BF16, tag="ot", name="ot",
                                       bufs=3)
                    nc.vector.tensor_copy(out=ot[:], in_=ps[:])
                    nc.sync.dma_start(
                        out=outT[n * P:(n + 1) * P,
                                 qh * 512:(qh + 1) * 512],
                        in_=ot[:])

                # ordered queue of keyed fill closures; dependents pull
                # their prerequisites forward so TensorE never deadlocks
                # on a not-yet-emitted producer
                fills = {}
                for sc in range(1, QC):
                    fills[("kt", 0, sc)] = (
                        lambda sc=sc: qk_group(kt_sb, wk_sb, 0, sc, False))
                for t in range(1, TC):
                    fills[("v", t)] = lambda t=t: v_group(t)
                for sc in range(1, QC):
                    fills[("qt", 0, sc)] = (
                        lambda sc=sc: qk_group(qt_sb, wq_sb, 0, sc, True))
                for sc in range(QC):
                    fills[("kt", 1, sc)] = (
                        lambda sc=sc: qk_group(kt_sb, wk_sb, 1, sc, False))
                for sc in range(QC):
                    fills[("qt", 1, sc)] = (
                        lambda sc=sc: qk_group(qt_sb, wq_sb, 1, sc, True))

                def run_fill(key=None):
                    """Emit one queued fill group (a specific one, or the
                    oldest)."""
                    if key is None:
                        if fills:
                            fills.pop(next(iter(fills)))()
                    elif key in fills:
                        fills.pop(key)()

                # ---- upfront: K(jc0,cols 0-1023), Q(jc0,cols 0-1023),
                # V(0) ----
                qk_group(kt_sb, wk_sb, 0, 0, False)
                qk_group(qt_sb, wq_sb, 0, 0, True)
                v_group(0)

                # ---- attention: head pairs x query chunks ----
                # Per key-chunk t, both heads' scores land in ONE [128,
                # 2x512] PSUM tile: head-even via PE row-tile T0 into the
                # first bank, head-odd via T8 into the second (row tiles
                # must hit different banks; adjacent T0/T8 matmuls overlap
                # ~2x). One plain-Exp ACT covers both heads, and the tile
                # is double-buffered so TensorE never waits on ScalarE.
                def ctx_mms(p, t, qh):
                    """Both heads' ctx accumulation for key chunk t."""
                    for half in range(2):
                        cp = (ctx_e if half == 0 else ctx_o)
                        nc.tensor.matmul(
                            cp[:],
                            lhsT=v_sb[:, t, 2 * p + half, :],
                            rhs=ets[t][:, half, :],
                            start=(t == 0), stop=(t == TC - 1),
                            skip_group_check=True,
                        )

                for p in range(NP if LVL >= 1 else 0):
                    for qh in range(QC):
                        # prerequisite of this iteration's score matmuls
                        run_fill(("qt", p, qh))
                        ctx_e = ps_ctx.tile([HD + 1, 512], F32, tag="ctx_e",
                                            name="ctx_e", bufs=1)
                        ctx_o = ps_ctx.tile([HD + 1, 512], F32, tag="ctx_o",
                                            name="ctx_o", bufs=1)
                        ets = {}
                        for tp in range(TC // 2):
                            # scores for key chunks (2tp, 2tp+1) back to
                            # back: all four matmuls alternate the T0/T8
                            # row-tiles and only then does the array switch
                            # back to full-128 mode for ctx/fill work
                            pss = {}
                            for t in (2 * tp, 2 * tp + 1):
                                run_fill(("kt", p, t // 4))
                                pss[t] = ps_sc.tile([P, 2, 512], F32,
                                                    tag="ps", name="ps",
                                                    bufs=2)
                            for t in (2 * tp, 2 * tp + 1):
                                for half in range(2):
                                    lo = 64 * half
                                    nc.tensor.matmul(
                                        pss[t][:, half, :],
                                        lhsT=kt_sb[lo:lo + HD, p,
                                                   t * P:(t + 1) * P],
                                        rhs=qt_sb[lo:lo + HD, p,
                                                  qh * 512:(qh + 1) * 512],
                                        start=True, stop=True,
                                        skip_group_check=True,
                                    )
                            if tp > 0 and LVL >= 2:
                                for t in (2 * tp - 2, 2 * tp - 1):
                                    run_fill(("v", t))
                                    ctx_mms(p, t, qh)
                            run_fill()
                            for t in (2 * tp, 2 * tp + 1):
                                et = et_pool.tile([P, 2, 512], BF16,
                                                  tag="et", name="et",
                                                  bufs=4)
                                nc.scalar.activation(et[:], pss[t][:], EXP)
                                ets[t] = et
                        if LVL >= 2:
                            for t in (TC - 2, TC - 1):
                                run_fill(("v", t))
                                ctx_mms(p, t, qh)
                        if LVL < 3:
                            continue
                        # normalize: ctx rows stay in PSUM; the sum rows
                        # are copied to SBUF (reciprocal_approx_fast cannot
                        # read PSUM), inverted, broadcast, multiplied in.
                        # Both heads' chains are interleaved so the DVE and
                        # gpsimd steps overlap across heads.
                        cps = (ctx_e, ctx_o)
                        srs, rrs, pbs = [], [], []
                        for half, cp in enumerate(cps):
                            sr = work_pool.tile([1, 512], F32, tag=f"sr{half}",
                                                name="sr", bufs=2)
                            nc.vector.tensor_copy(out=sr[:],
                                                  in_=cp[HD:HD + 1, :])
                            srs.append(sr)
                        for half, cp in enumerate(cps):
                            rr = work_pool.tile([1, 512], F32, tag=f"rr{half}",
                                                name="rr", bufs=2)
                            nc.vector.reciprocal_approx_fast(
                                out=rr[:], in_=srs[half][:])
                            rrs.append(rr)
                        for half, cp in enumerate(cps):
                            pb = work_pool.tile([HD, 512], F32, tag=f"pb{half}",
                                                name="pb", bufs=2)
                            nc.gpsimd.partition_broadcast(pb[:], rrs[half][:])
                            pbs.append(pb)
                        for half, cp in enumerate(cps):
                            lo = 64 * half
                            nc.vector.tensor_tensor(
                                out=ctn[lo:lo + HD, p,
                                        qh * 512:(qh + 1) * 512],
                                in0=cp[0:HD, :], in1=pbs[half][:], op=MULT)
                        if p == 1 and LVL >= 4:
                            for n in range(NC_O):
                                fills[("out", n, qh)] = (
                                    lambda n=n, qh=qh: out_group(n, qh))

                for key in list(fills):
                    fn = fills.pop(key)
                    if key[0] == "out":
                        out_group(key[1], key[2], tail=True)
                    else:
                        fn()

            if tiny_out:
                tt = out_pool.tile([P, 512], F32, name="tt", tag="tt", bufs=1)
                nc.vector.memset(tt[:], 1.0)
                nc.sync.dma_start(out=tiny[:], in_=tt[:])

    nc.compile()
    return nc


_NC_CACHE = {}


def get_nc(repeats: int = 1, tiny_out: bool = False):
    key = (repeats, tiny_out)
    if key not in _NC_CACHE:
        _NC_CACHE[key] = build_nc(repeats, tiny_out)
    return _NC_CACHE[key]


def host_prep(inputs):
    """Shard + precompute per-core input maps; return (in_maps, out_bias_row)."""
    f = {k: np.asarray(v, dtype=np.float64) for k, v in inputs.items()}
    x, env = f["x"], f["env_context"]
    Wq, Wk, Wv, Wo = f["Wq"], f["Wk"], f["Wv"], f["Wo"]
    bq, bk, bv, bo = f["bq"], f["bk"], f["bv"], f["bo"]
    We, be, Wm, bm = f["We"], f["be"], f["Wm"], f["bm"]

    gate = 1.0 / (1.0 + np.exp(-((env @ We + be) @ Wm + bm)))  # [B, H]
    scale = gate / np.sqrt(HD)                                  # [B, H]

    in_maps = []
    for c in range(N_CORES):
        b, g = divmod(c, 4)
        cols = slice(J * g, J * (g + 1))
        # fold gate/sqrt(HD) into this core's Wk columns (per-column scale)
        wk_scaled = Wk[:, cols] * np.repeat(
            scale[b, HPC * g:HPC * (g + 1)], HD)[None, :]
        wv_pad = np.zeros((D, HPC * (HD + 1)), np.float64)
        for i in range(HPC):
            h = HPC * g + i
            wv_pad[:, i * (HD + 1):i * (HD + 1) + HD] = \
                Wv[:, HD * h:HD * (h + 1)]
        in_maps.append({
            "xT": np.ascontiguousarray(x[b].T).astype(ml_dtypes.bfloat16),
            "wq": np.ascontiguousarray(Wq[:, cols]).astype(ml_dtypes.bfloat16),
            "wk": np.ascontiguousarray(wk_scaled).astype(ml_dtypes.bfloat16),
            "wv": wv_pad.astype(np.float32).astype(ml_dtypes.bfloat16),
            "wo": np.ascontiguousarray(
                Wo[J * g:J * (g + 1), :]).astype(ml_dtypes.bfloat16),
            "bqr": np.ascontiguousarray(
                bq[cols].reshape(JC, P).T).astype(np.float32),
        })
    out_bias_row = (bv @ Wo + bo).astype(np.float32)  # [D]
    return in_maps, out_bias_row


def assemble(results, out_bias_row):
    out = np.zeros((B, S, D), np.float32)
    for c in range(N_CORES):
        b = c // 4
        out[b] += results[c]["outT"].T.astype(np.float32)
    out += out_bias_row[None, None, :]
    return out


def kernel(**inputs):
    import time as _time

    nc = get_nc(1)
    in_maps, out_bias_row = host_prep(inputs)
    last_err = None
    for _attempt in range(4):
        try:
            res = bass_utils.run_bass_kernel_spmd(
                nc, in_maps, core_ids=list(range(N_CORES)))
            return assemble(res.results, out_bias_row)
        except Exception as e:  # transient NRT/axon hiccups recover on retry
            last_err = e
            _time.sleep(5)
    raise last_err
